# revision 15
# baseline (speedup 1.0000x reference)
"""Causal depthwise conv1d (K=3) + pointwise 1x1 conv for Trainium2.

Full-input contract: kernel(**inputs) takes the complete (unsharded) numpy
inputs and returns the complete output. Internally the work is sharded over
8 NeuronCores: core c handles batch b = c//2 and sequence half c%2
(L_chunk = 2048), with a (K-1)=2 column halo taken from the previous
sequence chunk (zeros at the causal left edge). The small conv weights are
replicated on every core.

All device I/O uses partition-major, tile-major contiguous DRAM layouts so
every DMA lowers to 128 large (4-24 KB) descriptors -- strided per-row
descriptors (~0.5-1 KB) measured 4-8x slower end-to-end.

Per-core compute layout is channel-major ([P=128 partitions, DC=8 chunks,
cols]). The depthwise conv per chunk: tap0 (+b_dw) on the scalar engine
(activation), taps 1+2 accumulate in-place into a bf16 y tile via DVE
scalar_tensor_tensor (STT has no packed 2x uop on trn2 -> ~1.04 ns/col
regardless of dtype, so the split ACT/DVE balances the two engines under
the PE's tile period). The pointwise conv is a bf16 K-contraction matmul;
PSUM is evacuated by ACT (fused + b_pw, bf16 out).

Schedule notes (from perfetto traces):
  - ~7us fixed framework preamble; first DMA issues right after it.
  - PE HAM clock gate: cold 1.2 GHz until ~3.4us of sustained matmul
    activity. A burst of dummy matmuls right after the preamble warms the
    PE while the first x tile + dw conv are still in flight.
  - Small first/last l-tiles shrink the pipeline fill/drain.
"""

import sys

if "/opt/trn_rl_repo" not in sys.path:
    sys.path.insert(0, "/opt/trn_rl_repo")

import numpy as np

import concourse.bass as bass
import concourse.tile as tile
from concourse import bacc, mybir
from concourse.bass_utils import run_bass_kernel_spmd

P = 128          # SBUF partitions
B, L, D = 4, 4096, 1024
KSZ = 3          # depthwise kernel taps
NCORES = 8
LC = (B * L) // NCORES   # 2048 sequence positions per core
PAD = 4          # 2 junk + 2 halo columns per chunk row (4B row alignment)
# l-tile schedule: small first tile shrinks the serial prologue (PE starts
# on the first columns early), small last tile shrinks the store drain.
LTS = [256, 512, 512, 512, 256]
assert sum(LTS) == LC
DC = D // P              # 8 channel chunks (contraction)
EC = D // P              # 8 output-channel chunks

NDUMMY = 6       # PE pre-warm matmuls (HAM un-throttle before first real MM)

# Per-chunk engine split for the depthwise conv (measured: ACTIVATE ~797ns,
# DVE STT ~737 (always 1x), DVE TS ~477 when 4B-aligned / ~737 misaligned,
# GpSimd TT ~742; all @512 cols). tap0 and t2 are aligned (DVE TS 2x);
# tap1 is 2B-misaligned (STT or ACT). GpSimd only adds pre-scaled terms.
ACT_TAP0 = (0, 1, 2, 3)        # tap0 on ACT; rest DVE tensor_scalar
GPS_TAP1 = (1, 3, 5, 7)        # tap1 = ACT-scaled t1 + GpSimd add
GPS_TAP2 = (0, 1, 2, 3, 4, 5)  # tap2 = DVE-TS t2 (2x) + GpSimd add
DVE_EVAC = (0, 4)              # evacs on DVE tensor_scalar (+b_pw)

_CACHED_NC = None

_LT_OFF = [0]
for _n in LTS:
    _LT_OFF.append(_LT_OFF[-1] + _n)
# flat column offset of each tile block in the packed x dram tensor
_XBLK = [0]
for _n in LTS:
    _XBLK.append(_XBLK[-1] + DC * (_n + PAD))


def _build_nc():
    nc = bacc.Bacc("TRN2", target_bir_lowering=False, debug=False,
                   num_devices=NCORES)
    f32 = mybir.dt.float32
    bf16 = mybir.dt.bfloat16

    # x packed per partition, tile-major: block lt = [DC, n+PAD] per
    # partition, cols 2..3 = halo, 4.. = data (tap k reads cols 2+k..)
    xt = nc.dram_tensor("xt", [P, _XBLK[-1]], bf16, kind="ExternalInput").ap()
    # weights packed per partition: wt[p, ec*1024 + dc*128 + j]
    #   = w_pw[ec*128+j, dc*128+p]
    wt = nc.dram_tensor("wt", [P, EC * DC * P], bf16,
                        kind="ExternalInput").ap()
    # per-channel params fp32, cols: w_dw[0..2], b_dw, b_pw
    pp = nc.dram_tensor("pp", [D, 5], f32, kind="ExternalInput").ap()
    # output packed per partition, tile-major: block lt = [EC, n]
    ot = nc.dram_tensor("ot", [P, EC * LC], bf16, kind="ExternalOutput").ap()

    pp_r = pp.rearrange("(o p) c -> p o c", p=P)    # [128, DC, 5]

    with tile.TileContext(nc) as tc:
        with (
            tc.tile_pool(name="wpool", bufs=1) as wpool,
            tc.tile_pool(name="ppool", bufs=1) as ppool,
            tc.tile_pool(name="dpool", bufs=1) as dpool,
            tc.tile_pool(name="xpool5", bufs=3) as xpool5,
            tc.tile_pool(name="xpool2", bufs=2) as xpool2,
            tc.tile_pool(name="ypool", bufs=18) as ypool,
            tc.tile_pool(name="tpool", bufs=6) as tpool,
            tc.tile_pool(name="opool5", bufs=2) as opool5,
            tc.tile_pool(name="opool2", bufs=2) as opool2,
            tc.tile_pool(name="psum", bufs=8, space="PSUM") as psum_pool,
        ):
            p_sb = ppool.tile([P, DC, 5], f32)
            w_sb0 = wpool.tile([P, 1 * DC * P], bf16, name="w_sb0")
            w_sb13 = wpool.tile([P, 3 * DC * P], bf16, name="w_sb13")
            w_sb47 = wpool.tile([P, 4 * DC * P], bf16, name="w_sb47")
            dummy_rhs = dpool.tile([P, 512], bf16, name="dummy_rhs")
            dummy_act = dpool.tile([P, 8], bf16, name="dummy_act")
            dummy_ps = psum_pool.tile([P, 512], f32, tag="acc",
                                      name="acc")

            def w_ap(ec, dc):
                if ec == 0:
                    return w_sb0[:, dc * P:(dc + 1) * P]
                if ec < 4:
                    return w_sb13[:, (ec - 1) * DC * P + dc * P:][:, :P]
                return w_sb47[:, (ec - 4) * DC * P + dc * P:][:, :P]

            def x_load(lt):
                """one contiguous DMA (SP queue) for l-tile lt"""
                n = LTS[lt]
                pool = xpool5 if n == 512 else xpool2
                xs = pool.tile([P, DC, n + PAD], bf16, tag=f"x{n}",
                               name="xs")
                nc.sync.dma_start(
                    xs[:],
                    xt[:, _XBLK[lt]:_XBLK[lt + 1]].rearrange(
                        "p (o c) -> p o c", c=n + PAD))
                return xs

            def dw_conv(lt, xs):
                """depthwise conv, spread over ACT/DVE/GpSimd per the
                ACT_TAP0/GPS_TAP1/GPS_TAP2 split"""
                n = LTS[lt]
                ys = []
                for dc in range(DC):
                    y = ypool.tile([P, 512], bf16, tag="y", name="y")[:, :n]
                    # tap0 (+b_dw) -> y
                    if dc in ACT_TAP0:
                        nc.scalar.activation(
                            y[:], xs[:, dc, 2:2 + n],
                            mybir.ActivationFunctionType.Identity,
                            bias=p_sb[:, dc, 3:4], scale=p_sb[:, dc, 0:1])
                    else:
                        nc.vector.tensor_scalar(
                            y[:], xs[:, dc, 2:2 + n],
                            p_sb[:, dc, 0:1], p_sb[:, dc, 3:4],
                            op0=mybir.AluOpType.mult,
                            op1=mybir.AluOpType.add)
                    # tap1 (misaligned)
                    if dc in GPS_TAP1:
                        t1 = tpool.tile([P, 512], bf16, tag="t1",
                                        name="t1")[:, :n]
                        nc.scalar.activation(
                            t1[:], xs[:, dc, 3:3 + n],
                            mybir.ActivationFunctionType.Identity,
                            bias=0.0, scale=p_sb[:, dc, 1:2])
                        nc.gpsimd.tensor_tensor(
                            y[:], y[:], t1[:], op=mybir.AluOpType.add)
                    else:
                        nc.vector.scalar_tensor_tensor(
                            y[:], xs[:, dc, 3:3 + n], p_sb[:, dc, 1:2], y[:],
                            op0=mybir.AluOpType.mult,
                            op1=mybir.AluOpType.add)
                    # tap2 (aligned)
                    if dc in GPS_TAP2:
                        t2 = tpool.tile([P, 512], bf16, tag="t2",
                                        name="t2")[:, :n]
                        nc.vector.tensor_scalar(
                            t2[:], xs[:, dc, 4:4 + n],
                            p_sb[:, dc, 2:3], 0.0,
                            op0=mybir.AluOpType.mult,
                            op1=mybir.AluOpType.add)
                        nc.gpsimd.tensor_tensor(
                            y[:], y[:], t2[:], op=mybir.AluOpType.add)
                    else:
                        nc.vector.scalar_tensor_tensor(
                            y[:], xs[:, dc, 4:4 + n], p_sb[:, dc, 2:3], y[:],
                            op0=mybir.AluOpType.mult,
                            op1=mybir.AluOpType.add)
                    ys.append(y)
                return ys

            def pointwise(lt, ys, o_sb, ecs):
                """o_sb[:, ec, :] = w_pw[ec] @ y + b_pw[ec] for e-chunks"""
                n = LTS[lt]
                for ec in ecs:
                    acc = psum_pool.tile([P, 512], f32, tag="acc",
                                         name="acc")[:, :n]
                    for dc in range(DC):
                        nc.tensor.matmul(
                            acc[:],
                            lhsT=w_ap(ec, dc),
                            rhs=ys[dc][:],
                            start=(dc == 0), stop=(dc == DC - 1))
                    if ec in DVE_EVAC:
                        nc.vector.tensor_scalar(
                            o_sb[:, ec, :], acc[:],
                            p_sb[:, ec, 4:5], 0.0,
                            op0=mybir.AluOpType.add,
                            op1=mybir.AluOpType.add)
                    else:
                        nc.scalar.activation(
                            o_sb[:, ec, :], acc[:],
                            mybir.ActivationFunctionType.Identity,
                            bias=p_sb[:, ec, 4:5], scale=1.0)

            def o_tile(lt):
                n = LTS[lt]
                pool = opool5 if n == 512 else opool2
                return pool.tile([P, EC, n], bf16, tag=f"o{n}", name="o_sb")

            def store_pair(lt, o_sb, i, eng):
                n = LTS[lt]
                s = _LT_OFF[lt]
                dst = ot[:, EC * s + 2 * i * n:EC * s + (2 * i + 2) * n]
                eng.dma_start(dst.rearrange("p (e c) -> p e c", c=n),
                              o_sb[:, 2 * i:2 * i + 2, :])

            def store(lt, o_sb):
                # per ec-pair on rotating queues: stores start draining as
                # soon as each pair is evacuated, and no DGE ring backs up
                for i, eng in enumerate([nc.gpsimd, nc.sync,
                                         nc.gpsimd, nc.sync]):
                    store_pair(lt, o_sb, i, eng)

            # --- emission (guides per-queue FIFO order) -----------------
            # warm-up: DVE memsets a junk rhs, ACT preloads its table, the
            # PE chews dummy matmuls so HAM un-throttles during the DMA wait
            nc.vector.memset(dummy_rhs[:], 0.0)
            nc.scalar.activation(
                dummy_act[:], dummy_rhs[:, 0:8],
                mybir.ActivationFunctionType.Identity, bias=0.0, scale=1.0)

            # everything the fill depends on rides the SP HWDGE queue --
            # the GpSimd SWDGE queue's first packet is ~4.5us late, so only
            # w47 (needed last) goes there
            nc.sync.dma_start(p_sb[:], pp_r[:])
            xs0 = x_load(0)
            nc.sync.dma_start(w_sb0[:], wt[:, 0:DC * P])

            for _ in range(NDUMMY):
                nc.tensor.matmul(dummy_ps[:], lhsT=dummy_rhs[:, 0:P],
                                 rhs=dummy_rhs[:], start=True, stop=True)

            xs1 = x_load(1)
            nc.sync.dma_start(w_sb13[:], wt[:, DC * P:4 * DC * P])
            ys0 = dw_conv(0, xs0)
            nc.gpsimd.dma_start(w_sb47[:], wt[:, 4 * DC * P:8 * DC * P])
            xs2 = x_load(2)
            ys1 = dw_conv(1, xs1)
            # pointwise(t) is emitted AFTER dw_conv(t+1): the evacuations
            # must sit BEHIND the next tile's conv taps in the ACT FIFO,
            # else ACT blocks on matmul-completion waits and starves DVE
            o0 = o_tile(0)
            pointwise(0, ys0, o0, range(EC))
            store(0, o0)

            xs3 = x_load(3)
            ys2 = dw_conv(2, xs2)
            o1 = o_tile(1)
            pointwise(1, ys1, o1, range(EC))
            store(1, o1)

            xs4 = x_load(4)
            ys3 = dw_conv(3, xs3)
            o2 = o_tile(2)
            pointwise(2, ys2, o2, range(EC))
            store(2, o2)

            ys4 = dw_conv(4, xs4)
            o3 = o_tile(3)
            pointwise(3, ys3, o3, range(EC))
            store(3, o3)

            # last tile: store per ec-pair, spread across the three DMA
            # queues so the final drain is short
            n4 = LTS[4]
            s4 = _LT_OFF[4]
            o4 = o_tile(4)
            last_q = [nc.sync, nc.gpsimd, nc.scalar, nc.sync]
            for i in range(4):
                pointwise(4, ys4, o4, range(2 * i, 2 * i + 2))
                dst = ot[:, EC * s4 + 2 * i * n4:EC * s4 + (2 * i + 2) * n4]
                last_q[i].dma_start(
                    dst.rearrange("p (e c) -> p e c", c=n4),
                    o4[:, 2 * i:2 * i + 2, :])

    nc.compile()  # bacc: legalizes multi-sem waits for TRN2 codegen
    return nc


def _shard_inputs(x, w_dw, b_dw, w_pw, b_pw):
    import ml_dtypes
    bf = ml_dtypes.bfloat16
    # wt[p, ec*1024 + dc*128 + j] = w_pw[ec*128+j, dc*128+p]
    wt = np.ascontiguousarray(
        w_pw.reshape(EC, P, DC, P).transpose(3, 0, 2, 1).reshape(P, -1)
    ).astype(bf)
    pp = np.ascontiguousarray(
        np.stack([w_dw[:, 0], w_dw[:, 1], w_dw[:, 2], b_dw, b_pw], axis=1),
        dtype=np.float32)                                        # (D, 5)
    in_maps = []
    for c in range(NCORES):
        b, half = divmod(c, 2)
        l0 = half * LC
        # xpad[d, t]: t 0..1 junk, 2..3 halo (x[l0-2], x[l0-1]), 4.. data
        xpad = np.zeros((D, LC + PAD), dtype=bf)
        lo = max(l0 - 2, 0)
        xpad[:, PAD - (l0 - lo):] = x[b, lo:l0 + LC, :].T.astype(bf)
        xtc = np.empty((P, _XBLK[-1]), dtype=bf)
        for lt, n in enumerate(LTS):
            s = _LT_OFF[lt]
            blk = xpad[:, s:s + n + PAD].reshape(DC, P, n + PAD)
            xtc[:, _XBLK[lt]:_XBLK[lt + 1]] = \
                blk.transpose(1, 0, 2).reshape(P, -1)
        in_maps.append({"xt": xtc, "wt": wt, "pp": pp})
    return in_maps


def kernel(x, w_dw, b_dw, w_pw, b_pw):
    assert x.shape == (B, L, D) and w_dw.shape == (D, KSZ)
    global _CACHED_NC
    if _CACHED_NC is None:
        _CACHED_NC = _build_nc()
    in_maps = _shard_inputs(np.asarray(x, dtype=np.float32),
                            np.asarray(w_dw), np.asarray(b_dw),
                            np.asarray(w_pw), np.asarray(b_pw))
    results = run_bass_kernel_spmd(
        _CACHED_NC, in_maps, list(range(NCORES))).results
    out = np.empty((B, L, D), dtype=np.float32)
    for c in range(NCORES):
        b, half = divmod(c, 2)
        l0 = half * LC
        o = results[c]["ot"]
        for lt, n in enumerate(LTS):
            s = _LT_OFF[lt]
            blk = o[:, EC * s:EC * (s + n)].reshape(P, EC, n)
            out[b, l0 + s:l0 + s + n, :] = \
                blk.transpose(2, 1, 0).reshape(n, D).astype(np.float32)
    return out


# revision 16
# speedup vs baseline: 1.0236x; 1.0236x over previous
"""Causal depthwise conv1d (K=3) + pointwise 1x1 conv for Trainium2.

Full-input contract: kernel(**inputs) takes the complete (unsharded) numpy
inputs and returns the complete output. Internally the work is sharded over
8 NeuronCores: core c handles batch b = c//2 and sequence half c%2
(L_chunk = 2048), with a (K-1)=2 column halo taken from the previous
sequence chunk (zeros at the causal left edge). The small conv weights are
replicated on every core.

All device I/O uses partition-major, tile-major contiguous DRAM layouts so
every DMA lowers to 128 large (4-24 KB) descriptors -- strided per-row
descriptors (~0.5-1 KB) measured 4-8x slower end-to-end.

Per-core compute layout is channel-major ([P=128 partitions, DC=8 chunks,
cols]). The depthwise conv per chunk: tap0 (+b_dw) on the scalar engine
(activation), taps 1+2 accumulate in-place into a bf16 y tile via DVE
scalar_tensor_tensor (STT has no packed 2x uop on trn2 -> ~1.04 ns/col
regardless of dtype, so the split ACT/DVE balances the two engines under
the PE's tile period). The pointwise conv is a bf16 K-contraction matmul;
PSUM is evacuated by ACT (fused + b_pw, bf16 out).

Schedule notes (from perfetto traces):
  - ~7us fixed framework preamble; first DMA issues right after it.
  - PE HAM clock gate: cold 1.2 GHz until ~3.4us of sustained matmul
    activity. A burst of dummy matmuls right after the preamble warms the
    PE while the first x tile + dw conv are still in flight.
  - Small first/last l-tiles shrink the pipeline fill/drain.
"""

import sys

if "/opt/trn_rl_repo" not in sys.path:
    sys.path.insert(0, "/opt/trn_rl_repo")

import numpy as np

import concourse.bass as bass
import concourse.tile as tile
from concourse import bacc, mybir
from concourse.bass_utils import run_bass_kernel_spmd

P = 128          # SBUF partitions
B, L, D = 4, 4096, 1024
KSZ = 3          # depthwise kernel taps
NCORES = 8
LC = (B * L) // NCORES   # 2048 sequence positions per core
PAD = 4          # 2 junk + 2 halo columns per chunk row (4B row alignment)
# l-tile schedule: small first tile shrinks the serial prologue (PE starts
# on the first columns early), small last tile shrinks the store drain.
LTS = [256, 512, 512, 512, 256]
assert sum(LTS) == LC
DC = D // P              # 8 channel chunks (contraction)
EC = D // P              # 8 output-channel chunks

NDUMMY = 6       # PE pre-warm matmuls (HAM un-throttle before first real MM)

# Per-chunk engine split for the depthwise conv (measured: ACTIVATE ~797ns,
# DVE STT ~737 (always 1x), DVE TS ~477 when 4B-aligned / ~737 misaligned,
# GpSimd TT ~742; all @512 cols). tap0 and t2 are aligned (DVE TS 2x);
# tap1 is 2B-misaligned (STT or ACT). GpSimd only adds pre-scaled terms.
ACT_TAP0 = (0, 1, 2, 3, 4, 5)  # tap0 on ACT; rest DVE tensor_scalar
GPS_TAP1 = (1, 3, 5)           # tap1 = ACT-scaled t1 + GpSimd add
GPS_TAP2 = (2, 6)              # tap2 = DVE-TS t2 (2x) + GpSimd add
DVE_EVAC = (0, 4)              # evacs on DVE tensor_scalar (+b_pw)

_CACHED_NC = None

_LT_OFF = [0]
for _n in LTS:
    _LT_OFF.append(_LT_OFF[-1] + _n)
# flat column offset of each tile block in the packed x dram tensor
_XBLK = [0]
for _n in LTS:
    _XBLK.append(_XBLK[-1] + DC * (_n + PAD))


def _build_nc():
    nc = bacc.Bacc("TRN2", target_bir_lowering=False, debug=False,
                   num_devices=NCORES)
    f32 = mybir.dt.float32
    bf16 = mybir.dt.bfloat16

    # x packed per partition, tile-major: block lt = [DC, n+PAD] per
    # partition, cols 2..3 = halo, 4.. = data (tap k reads cols 2+k..)
    xt = nc.dram_tensor("xt", [P, _XBLK[-1]], bf16, kind="ExternalInput").ap()
    # weights packed per partition: wt[p, ec*1024 + dc*128 + j]
    #   = w_pw[ec*128+j, dc*128+p]
    wt = nc.dram_tensor("wt", [P, EC * DC * P], bf16,
                        kind="ExternalInput").ap()
    # per-channel params fp32 packed per partition: pp[p, dc*5+c],
    # cols c: w_dw[0..2], b_dw, b_pw  (128 x 40B descriptors, not 1024 x 20B)
    pp = nc.dram_tensor("pp", [P, DC * 5], f32, kind="ExternalInput").ap()
    # output packed per partition, tile-major: block lt = [EC, n]
    ot = nc.dram_tensor("ot", [P, EC * LC], bf16, kind="ExternalOutput").ap()


    with tile.TileContext(nc) as tc:
        with (
            tc.tile_pool(name="wpool", bufs=1) as wpool,
            tc.tile_pool(name="ppool", bufs=1) as ppool,
            tc.tile_pool(name="dpool", bufs=1) as dpool,
            tc.tile_pool(name="xpool5", bufs=3) as xpool5,
            tc.tile_pool(name="xpool2", bufs=2) as xpool2,
            tc.tile_pool(name="ypool", bufs=18) as ypool,
            tc.tile_pool(name="tpool", bufs=6) as tpool,
            tc.tile_pool(name="opool5", bufs=2) as opool5,
            tc.tile_pool(name="opool2", bufs=2) as opool2,
            tc.tile_pool(name="psum", bufs=8, space="PSUM") as psum_pool,
        ):
            p_sb = ppool.tile([P, DC, 5], f32)
            w_sb0 = wpool.tile([P, 1 * DC * P], bf16, name="w_sb0")
            w_sb13 = wpool.tile([P, 3 * DC * P], bf16, name="w_sb13")
            w_sb47 = wpool.tile([P, 4 * DC * P], bf16, name="w_sb47")
            dummy_rhs = dpool.tile([P, 512], bf16, name="dummy_rhs")
            dummy_act = dpool.tile([P, 8], bf16, name="dummy_act")
            dummy_ps = psum_pool.tile([P, 512], f32, tag="acc",
                                      name="acc")

            def w_ap(ec, dc):
                if ec == 0:
                    return w_sb0[:, dc * P:(dc + 1) * P]
                if ec < 4:
                    return w_sb13[:, (ec - 1) * DC * P + dc * P:][:, :P]
                return w_sb47[:, (ec - 4) * DC * P + dc * P:][:, :P]

            def x_load(lt):
                """one contiguous DMA (SP queue) for l-tile lt"""
                n = LTS[lt]
                pool = xpool5 if n == 512 else xpool2
                xs = pool.tile([P, DC, n + PAD], bf16, tag=f"x{n}",
                               name="xs")
                nc.sync.dma_start(
                    xs[:],
                    xt[:, _XBLK[lt]:_XBLK[lt + 1]].rearrange(
                        "p (o c) -> p o c", c=n + PAD))
                return xs

            def dw_conv(lt, xs):
                """depthwise conv, spread over ACT/DVE/GpSimd per the
                ACT_TAP0/GPS_TAP1/GPS_TAP2 split"""
                n = LTS[lt]
                ys = []
                for dc in range(DC):
                    y = ypool.tile([P, 512], bf16, tag="y", name="y")[:, :n]
                    # tap0 (+b_dw) -> y
                    if dc in ACT_TAP0:
                        nc.scalar.activation(
                            y[:], xs[:, dc, 2:2 + n],
                            mybir.ActivationFunctionType.Identity,
                            bias=p_sb[:, dc, 3:4], scale=p_sb[:, dc, 0:1])
                    else:
                        nc.vector.tensor_scalar(
                            y[:], xs[:, dc, 2:2 + n],
                            p_sb[:, dc, 0:1], p_sb[:, dc, 3:4],
                            op0=mybir.AluOpType.mult,
                            op1=mybir.AluOpType.add)
                    # tap1 (misaligned)
                    if dc in GPS_TAP1:
                        t1 = tpool.tile([P, 512], bf16, tag="t1",
                                        name="t1")[:, :n]
                        nc.scalar.activation(
                            t1[:], xs[:, dc, 3:3 + n],
                            mybir.ActivationFunctionType.Identity,
                            bias=0.0, scale=p_sb[:, dc, 1:2])
                        nc.gpsimd.tensor_tensor(
                            y[:], y[:], t1[:], op=mybir.AluOpType.add)
                    else:
                        nc.vector.scalar_tensor_tensor(
                            y[:], xs[:, dc, 3:3 + n], p_sb[:, dc, 1:2], y[:],
                            op0=mybir.AluOpType.mult,
                            op1=mybir.AluOpType.add)
                    # tap2 (aligned)
                    if dc in GPS_TAP2:
                        t2 = tpool.tile([P, 512], bf16, tag="t2",
                                        name="t2")[:, :n]
                        nc.vector.tensor_scalar(
                            t2[:], xs[:, dc, 4:4 + n],
                            p_sb[:, dc, 2:3], 0.0,
                            op0=mybir.AluOpType.mult,
                            op1=mybir.AluOpType.add)
                        nc.gpsimd.tensor_tensor(
                            y[:], y[:], t2[:], op=mybir.AluOpType.add)
                    else:
                        nc.vector.scalar_tensor_tensor(
                            y[:], xs[:, dc, 4:4 + n], p_sb[:, dc, 2:3], y[:],
                            op0=mybir.AluOpType.mult,
                            op1=mybir.AluOpType.add)
                    ys.append(y)
                return ys

            def pointwise(lt, ys, o_sb, ecs):
                """o_sb[:, ec, :] = w_pw[ec] @ y + b_pw[ec] for e-chunks"""
                n = LTS[lt]
                for ec in ecs:
                    acc = psum_pool.tile([P, 512], f32, tag="acc",
                                         name="acc")[:, :n]
                    for dc in range(DC):
                        nc.tensor.matmul(
                            acc[:],
                            lhsT=w_ap(ec, dc),
                            rhs=ys[dc][:],
                            start=(dc == 0), stop=(dc == DC - 1))
                    if ec in DVE_EVAC:
                        nc.vector.tensor_scalar(
                            o_sb[:, ec, :], acc[:],
                            p_sb[:, ec, 4:5], 0.0,
                            op0=mybir.AluOpType.add,
                            op1=mybir.AluOpType.add)
                    else:
                        nc.scalar.activation(
                            o_sb[:, ec, :], acc[:],
                            mybir.ActivationFunctionType.Identity,
                            bias=p_sb[:, ec, 4:5], scale=1.0)

            def o_tile(lt):
                n = LTS[lt]
                pool = opool5 if n == 512 else opool2
                return pool.tile([P, EC, n], bf16, tag=f"o{n}", name="o_sb")

            def store_pair(lt, o_sb, i, eng):
                n = LTS[lt]
                s = _LT_OFF[lt]
                dst = ot[:, EC * s + 2 * i * n:EC * s + (2 * i + 2) * n]
                eng.dma_start(dst.rearrange("p (e c) -> p e c", c=n),
                              o_sb[:, 2 * i:2 * i + 2, :])

            def store(lt, o_sb):
                # per ec-pair on rotating queues: stores start draining as
                # soon as each pair is evacuated, and no DGE ring backs up
                for i, eng in enumerate([nc.gpsimd, nc.sync,
                                         nc.gpsimd, nc.sync]):
                    store_pair(lt, o_sb, i, eng)

            # --- emission (guides per-queue FIFO order) -----------------
            # warm-up: DVE memsets a junk rhs, ACT preloads its table, the
            # PE chews dummy matmuls so HAM un-throttles during the DMA wait
            nc.vector.memset(dummy_rhs[:], 0.0)
            nc.scalar.activation(
                dummy_act[:], dummy_rhs[:, 0:8],
                mybir.ActivationFunctionType.Identity, bias=0.0, scale=1.0)

            # everything the fill depends on rides the SP HWDGE queue --
            # the GpSimd SWDGE queue's first packet is ~4.5us late, so only
            # w47 (needed last) goes there
            nc.sync.dma_start(p_sb[:],
                              pp.rearrange("p (o c) -> p o c", c=5))
            xs0 = x_load(0)
            nc.sync.dma_start(w_sb0[:], wt[:, 0:DC * P])

            for _ in range(NDUMMY):
                nc.tensor.matmul(dummy_ps[:], lhsT=dummy_rhs[:, 0:P],
                                 rhs=dummy_rhs[:], start=True, stop=True)

            xs1 = x_load(1)
            nc.sync.dma_start(w_sb13[:], wt[:, DC * P:4 * DC * P])
            ys0 = dw_conv(0, xs0)
            nc.gpsimd.dma_start(w_sb47[:], wt[:, 4 * DC * P:8 * DC * P])
            xs2 = x_load(2)
            ys1 = dw_conv(1, xs1)
            # pointwise(t) is emitted AFTER dw_conv(t+1): the evacuations
            # must sit BEHIND the next tile's conv taps in the ACT FIFO,
            # else ACT blocks on matmul-completion waits and starves DVE
            o0 = o_tile(0)
            pointwise(0, ys0, o0, range(EC))
            store(0, o0)

            xs3 = x_load(3)
            ys2 = dw_conv(2, xs2)
            o1 = o_tile(1)
            pointwise(1, ys1, o1, range(EC))
            store(1, o1)

            xs4 = x_load(4)
            ys3 = dw_conv(3, xs3)
            o2 = o_tile(2)
            pointwise(2, ys2, o2, range(EC))
            store(2, o2)

            ys4 = dw_conv(4, xs4)
            o3 = o_tile(3)
            pointwise(3, ys3, o3, range(EC))
            store(3, o3)

            # last tile: store per ec-pair, spread across the three DMA
            # queues so the final drain is short
            n4 = LTS[4]
            s4 = _LT_OFF[4]
            o4 = o_tile(4)
            last_q = [nc.sync, nc.gpsimd, nc.scalar, nc.sync]
            for i in range(4):
                pointwise(4, ys4, o4, range(2 * i, 2 * i + 2))
                dst = ot[:, EC * s4 + 2 * i * n4:EC * s4 + (2 * i + 2) * n4]
                last_q[i].dma_start(
                    dst.rearrange("p (e c) -> p e c", c=n4),
                    o4[:, 2 * i:2 * i + 2, :])

    nc.compile()  # bacc: legalizes multi-sem waits for TRN2 codegen
    return nc


def _shard_inputs(x, w_dw, b_dw, w_pw, b_pw):
    import ml_dtypes
    bf = ml_dtypes.bfloat16
    # wt[p, ec*1024 + dc*128 + j] = w_pw[ec*128+j, dc*128+p]
    wt = np.ascontiguousarray(
        w_pw.reshape(EC, P, DC, P).transpose(3, 0, 2, 1).reshape(P, -1)
    ).astype(bf)
    pp = np.ascontiguousarray(
        np.stack([w_dw[:, 0], w_dw[:, 1], w_dw[:, 2], b_dw, b_pw], axis=1)
        .astype(np.float32).reshape(DC, P, 5).transpose(1, 0, 2)
        .reshape(P, DC * 5))                                     # (P, 40)
    in_maps = []
    for c in range(NCORES):
        b, half = divmod(c, 2)
        l0 = half * LC
        # xpad[d, t]: t 0..1 junk, 2..3 halo (x[l0-2], x[l0-1]), 4.. data
        xpad = np.zeros((D, LC + PAD), dtype=bf)
        lo = max(l0 - 2, 0)
        xpad[:, PAD - (l0 - lo):] = x[b, lo:l0 + LC, :].T.astype(bf)
        xtc = np.empty((P, _XBLK[-1]), dtype=bf)
        for lt, n in enumerate(LTS):
            s = _LT_OFF[lt]
            blk = xpad[:, s:s + n + PAD].reshape(DC, P, n + PAD)
            xtc[:, _XBLK[lt]:_XBLK[lt + 1]] = \
                blk.transpose(1, 0, 2).reshape(P, -1)
        in_maps.append({"xt": xtc, "wt": wt, "pp": pp})
    return in_maps


def kernel(x, w_dw, b_dw, w_pw, b_pw):
    assert x.shape == (B, L, D) and w_dw.shape == (D, KSZ)
    global _CACHED_NC
    if _CACHED_NC is None:
        _CACHED_NC = _build_nc()
    in_maps = _shard_inputs(np.asarray(x, dtype=np.float32),
                            np.asarray(w_dw), np.asarray(b_dw),
                            np.asarray(w_pw), np.asarray(b_pw))
    results = run_bass_kernel_spmd(
        _CACHED_NC, in_maps, list(range(NCORES))).results
    out = np.empty((B, L, D), dtype=np.float32)
    for c in range(NCORES):
        b, half = divmod(c, 2)
        l0 = half * LC
        o = results[c]["ot"]
        for lt, n in enumerate(LTS):
            s = _LT_OFF[lt]
            blk = o[:, EC * s:EC * (s + n)].reshape(P, EC, n)
            out[b, l0 + s:l0 + s + n, :] = \
                blk.transpose(2, 1, 0).reshape(n, D).astype(np.float32)
    return out


# revision 17
# speedup vs baseline: 1.1049x; 1.0794x over previous
"""Causal depthwise conv1d (K=3) + pointwise 1x1 conv for Trainium2.

Full-input contract: kernel(**inputs) takes the complete (unsharded) numpy
inputs and returns the complete output. Internally the work is sharded over
8 NeuronCores: core c handles batch b = c//2 and sequence half c%2
(L_chunk = 2048), with a (K-1)=2 column halo taken from the previous
sequence chunk (zeros at the causal left edge). The small conv weights are
replicated on every core.

All device I/O uses partition-major, tile-major contiguous DRAM layouts so
every DMA lowers to 128 large (4-24 KB) descriptors -- strided per-row
descriptors (~0.5-1 KB) measured 4-8x slower end-to-end.

Per-core compute layout is channel-major ([P=128 partitions, DC=8 chunks,
cols]). The depthwise conv per chunk: tap0 (+b_dw) on the scalar engine
(activation), taps 1+2 accumulate in-place into a bf16 y tile via DVE
scalar_tensor_tensor (STT has no packed 2x uop on trn2 -> ~1.04 ns/col
regardless of dtype, so the split ACT/DVE balances the two engines under
the PE's tile period). The pointwise conv is a bf16 K-contraction matmul;
PSUM is evacuated by ACT (fused + b_pw, bf16 out).

Schedule notes (from perfetto traces):
  - ~7us fixed framework preamble; first DMA issues right after it.
  - PE HAM clock gate: cold 1.2 GHz until ~3.4us of sustained matmul
    activity. A burst of dummy matmuls right after the preamble warms the
    PE while the first x tile + dw conv are still in flight.
  - Small first/last l-tiles shrink the pipeline fill/drain.
"""

import sys

if "/opt/trn_rl_repo" not in sys.path:
    sys.path.insert(0, "/opt/trn_rl_repo")

import numpy as np

import concourse.bass as bass
import concourse.tile as tile
from concourse import bacc, mybir
from concourse.bass_utils import run_bass_kernel_spmd

P = 128          # SBUF partitions
B, L, D = 4, 4096, 1024
KSZ = 3          # depthwise kernel taps
NCORES = 8
LC = (B * L) // NCORES   # 2048 sequence positions per core
PAD = 4          # 2 junk + 2 halo columns per chunk row (4B row alignment)
# l-tile schedule: small first tile shrinks the serial prologue (PE starts
# on the first columns early), small last tile shrinks the store drain.
LTS = [256, 512, 512, 512, 256]
assert sum(LTS) == LC
DC = D // P              # 8 channel chunks (contraction)
EC = D // P              # 8 output-channel chunks

NDUMMY = 6       # PE pre-warm matmuls (HAM un-throttle before first real MM)

# Per-chunk engine split for the depthwise conv (measured: ACTIVATE ~797ns,
# DVE STT ~737 (always 1x), DVE TS ~477 when 4B-aligned / ~737 misaligned,
# GpSimd TT ~742; all @512 cols). tap0 and t2 are aligned (DVE TS 2x);
# tap1 is 2B-misaligned (STT or ACT). GpSimd only adds pre-scaled terms.
ACT_TAP0 = (0, 1, 2, 3, 4, 5)  # tap0 on ACT; rest DVE tensor_scalar
GPS_TAP1 = (1, 3, 5)           # tap1 = ACT-scaled t1 + GpSimd add
GPS_TAP2 = (2, 6)              # tap2 = DVE-TS t2 (2x) + GpSimd add
DVE_EVAC = (0, 4)              # evacs on DVE tensor_scalar (+b_pw)

_CACHED_NC = None

_LT_OFF = [0]
for _n in LTS:
    _LT_OFF.append(_LT_OFF[-1] + _n)
# flat column offset of each tile block in the packed x dram tensor
_XBLK = [0]
for _n in LTS:
    _XBLK.append(_XBLK[-1] + DC * (_n + PAD))


def _build_nc():
    nc = bacc.Bacc("TRN2", target_bir_lowering=False, debug=False,
                   num_devices=NCORES)
    f32 = mybir.dt.float32
    bf16 = mybir.dt.bfloat16

    # x packed per partition, tile-major: block lt = [DC, n+PAD] per
    # partition, cols 2..3 = halo, 4.. = data (tap k reads cols 2+k..)
    xt = nc.dram_tensor("xt", [P, _XBLK[-1]], bf16, kind="ExternalInput").ap()
    # weights packed per partition: wt[p, ec*1024 + dc*128 + j]
    #   = w_pw[ec*128+j, dc*128+p]
    wt = nc.dram_tensor("wt", [P, EC * DC * P], bf16,
                        kind="ExternalInput").ap()
    # per-channel params fp32 packed per partition: pp[p, dc*5+c],
    # cols c: w_dw[0..2], b_dw, b_pw  (128 x 40B descriptors, not 1024 x 20B)
    pp = nc.dram_tensor("pp", [P, DC * 5], f32, kind="ExternalInput").ap()
    # output packed per partition, tile-major: block lt = [EC, n]
    ot = nc.dram_tensor("ot", [P, EC * LC], bf16, kind="ExternalOutput").ap()


    with tile.TileContext(nc) as tc:
        with (
            tc.tile_pool(name="wpool", bufs=1) as wpool,
            tc.tile_pool(name="ppool", bufs=1) as ppool,
            tc.tile_pool(name="dpool", bufs=1) as dpool,
            tc.tile_pool(name="xpool5", bufs=3) as xpool5,
            tc.tile_pool(name="xpool2", bufs=2) as xpool2,
            tc.tile_pool(name="ypool", bufs=18) as ypool,
            tc.tile_pool(name="tpool", bufs=6) as tpool,
            tc.tile_pool(name="opool5", bufs=2) as opool5,
            tc.tile_pool(name="opool2", bufs=2) as opool2,
            tc.tile_pool(name="psum", bufs=8, space="PSUM") as psum_pool,
        ):
            p_sb = ppool.tile([P, DC, 5], f32)
            w_sb0 = wpool.tile([P, 1 * DC * P], bf16, name="w_sb0")
            w_sb13 = wpool.tile([P, 3 * DC * P], bf16, name="w_sb13")
            w_sb47 = wpool.tile([P, 4 * DC * P], bf16, name="w_sb47")
            dummy_rhs = dpool.tile([P, 512], bf16, name="dummy_rhs")
            dummy_act = dpool.tile([P, 8], bf16, name="dummy_act")
            dummy_ps = psum_pool.tile([P, 512], f32, tag="acc",
                                      name="acc")

            def w_ap(ec, dc):
                if ec == 0:
                    return w_sb0[:, dc * P:(dc + 1) * P]
                if ec < 4:
                    return w_sb13[:, (ec - 1) * DC * P + dc * P:][:, :P]
                return w_sb47[:, (ec - 4) * DC * P + dc * P:][:, :P]

            def x_load(lt, nsplit=1):
                """contiguous DMA(s) (SP queue) for l-tile lt; nsplit>1
                slices it per chunk-group so the first chunks land early
                (the first big transfer runs well below fabric rate)"""
                n = LTS[lt]
                pool = xpool5 if n == 512 else xpool2
                xs = pool.tile([P, DC, n + PAD], bf16, tag=f"x{n}",
                               name="xs")
                w = DC // nsplit
                for i in range(nsplit):
                    b0 = _XBLK[lt] + i * w * (n + PAD)
                    nc.sync.dma_start(
                        xs[:, i * w:(i + 1) * w, :],
                        xt[:, b0:b0 + w * (n + PAD)].rearrange(
                            "p (o c) -> p o c", c=n + PAD))
                return xs

            def dw_conv(lt, xs):
                """depthwise conv, spread over ACT/DVE/GpSimd per the
                ACT_TAP0/GPS_TAP1/GPS_TAP2 split. GpSimd adds are emitted
                strictly LAST per chunk: a DVE op ordered after a GpSimd op
                on the same y would block the (strict-FIFO) DVE queue on
                GpSimd latency."""
                n = LTS[lt]
                ys = []
                for dc in range(DC):
                    y = ypool.tile([P, 512], bf16, tag="y", name="y")[:, :n]
                    prods = []
                    if dc in GPS_TAP2:
                        t2 = tpool.tile([P, 512], bf16, tag="t2",
                                        name="t2")[:, :n]
                        nc.vector.tensor_scalar(
                            t2[:], xs[:, dc, 4:4 + n],
                            p_sb[:, dc, 2:3], 0.0,
                            op0=mybir.AluOpType.mult,
                            op1=mybir.AluOpType.add)
                        prods.append(t2)
                    # tap0 (+b_dw) -> y
                    if dc in ACT_TAP0:
                        nc.scalar.activation(
                            y[:], xs[:, dc, 2:2 + n],
                            mybir.ActivationFunctionType.Identity,
                            bias=p_sb[:, dc, 3:4], scale=p_sb[:, dc, 0:1])
                    else:
                        nc.vector.tensor_scalar(
                            y[:], xs[:, dc, 2:2 + n],
                            p_sb[:, dc, 0:1], p_sb[:, dc, 3:4],
                            op0=mybir.AluOpType.mult,
                            op1=mybir.AluOpType.add)
                    if dc in GPS_TAP1:
                        t1 = tpool.tile([P, 512], bf16, tag="t1",
                                        name="t1")[:, :n]
                        nc.scalar.activation(
                            t1[:], xs[:, dc, 3:3 + n],
                            mybir.ActivationFunctionType.Identity,
                            bias=0.0, scale=p_sb[:, dc, 1:2])
                        prods.append(t1)
                    else:
                        nc.vector.scalar_tensor_tensor(
                            y[:], xs[:, dc, 3:3 + n], p_sb[:, dc, 1:2], y[:],
                            op0=mybir.AluOpType.mult,
                            op1=mybir.AluOpType.add)
                    if dc not in GPS_TAP2:
                        nc.vector.scalar_tensor_tensor(
                            y[:], xs[:, dc, 4:4 + n], p_sb[:, dc, 2:3], y[:],
                            op0=mybir.AluOpType.mult,
                            op1=mybir.AluOpType.add)
                    for t in prods:
                        nc.gpsimd.tensor_tensor(
                            y[:], y[:], t[:], op=mybir.AluOpType.add)
                    ys.append(y)
                return ys

            def pointwise(lt, ys, o_sb, ecs):
                """o_sb[:, ec, :] = w_pw[ec] @ y + b_pw[ec] for e-chunks"""
                n = LTS[lt]
                for ec in ecs:
                    acc = psum_pool.tile([P, 512], f32, tag="acc",
                                         name="acc")[:, :n]
                    for dc in range(DC):
                        nc.tensor.matmul(
                            acc[:],
                            lhsT=w_ap(ec, dc),
                            rhs=ys[dc][:],
                            start=(dc == 0), stop=(dc == DC - 1))
                    if ec in DVE_EVAC:
                        nc.vector.tensor_scalar(
                            o_sb[:, ec, :], acc[:],
                            p_sb[:, ec, 4:5], 0.0,
                            op0=mybir.AluOpType.add,
                            op1=mybir.AluOpType.add)
                    else:
                        nc.scalar.activation(
                            o_sb[:, ec, :], acc[:],
                            mybir.ActivationFunctionType.Identity,
                            bias=p_sb[:, ec, 4:5], scale=1.0)

            def o_tile(lt):
                n = LTS[lt]
                pool = opool5 if n == 512 else opool2
                return pool.tile([P, EC, n], bf16, tag=f"o{n}", name="o_sb")

            def store_pair(lt, o_sb, i, eng):
                n = LTS[lt]
                s = _LT_OFF[lt]
                dst = ot[:, EC * s + 2 * i * n:EC * s + (2 * i + 2) * n]
                eng.dma_start(dst.rearrange("p (e c) -> p e c", c=n),
                              o_sb[:, 2 * i:2 * i + 2, :])

            def store(lt, o_sb):
                # per ec-pair on rotating queues: stores start draining as
                # soon as each pair is evacuated, and no DGE ring backs up
                for i, eng in enumerate([nc.gpsimd, nc.sync,
                                         nc.gpsimd, nc.sync]):
                    store_pair(lt, o_sb, i, eng)

            # --- emission (guides per-queue FIFO order) -----------------
            # warm-up: DVE memsets a junk rhs, ACT preloads its table, the
            # PE chews dummy matmuls so HAM un-throttles during the DMA wait
            nc.vector.memset(dummy_rhs[:], 0.0)
            nc.scalar.activation(
                dummy_act[:], dummy_rhs[:, 0:8],
                mybir.ActivationFunctionType.Identity, bias=0.0, scale=1.0)

            # everything the fill depends on rides the SP HWDGE queue --
            # the GpSimd SWDGE queue's first packet is ~4.5us late, so only
            # w47 (needed last) goes there
            nc.sync.dma_start(p_sb[:],
                              pp.rearrange("p (o c) -> p o c", c=5))
            xs0 = x_load(0, nsplit=4)
            nc.sync.dma_start(w_sb0[:], wt[:, 0:DC * P])

            for _ in range(NDUMMY):
                nc.tensor.matmul(dummy_ps[:], lhsT=dummy_rhs[:, 0:P],
                                 rhs=dummy_rhs[:], start=True, stop=True)

            xs1 = x_load(1)
            nc.sync.dma_start(w_sb13[:], wt[:, DC * P:4 * DC * P])
            ys0 = dw_conv(0, xs0)
            nc.gpsimd.dma_start(w_sb47[:], wt[:, 4 * DC * P:8 * DC * P])
            xs2 = x_load(2)
            ys1 = dw_conv(1, xs1)
            # pointwise(t) is emitted AFTER dw_conv(t+1): the evacuations
            # must sit BEHIND the next tile's conv taps in the ACT FIFO,
            # else ACT blocks on matmul-completion waits and starves DVE
            o0 = o_tile(0)
            pointwise(0, ys0, o0, range(EC))
            store(0, o0)

            xs3 = x_load(3)
            ys2 = dw_conv(2, xs2)
            o1 = o_tile(1)
            pointwise(1, ys1, o1, range(EC))
            store(1, o1)

            xs4 = x_load(4)
            ys3 = dw_conv(3, xs3)
            o2 = o_tile(2)
            pointwise(2, ys2, o2, range(EC))
            store(2, o2)

            ys4 = dw_conv(4, xs4)
            o3 = o_tile(3)
            pointwise(3, ys3, o3, range(EC))
            store(3, o3)

            # last tile: store per ec-pair, spread across the three DMA
            # queues so the final drain is short
            n4 = LTS[4]
            s4 = _LT_OFF[4]
            o4 = o_tile(4)
            last_q = [nc.sync, nc.gpsimd, nc.scalar, nc.sync]
            for i in range(4):
                pointwise(4, ys4, o4, range(2 * i, 2 * i + 2))
                dst = ot[:, EC * s4 + 2 * i * n4:EC * s4 + (2 * i + 2) * n4]
                last_q[i].dma_start(
                    dst.rearrange("p (e c) -> p e c", c=n4),
                    o4[:, 2 * i:2 * i + 2, :])

    nc.compile()  # bacc: legalizes multi-sem waits for TRN2 codegen
    return nc


def _shard_inputs(x, w_dw, b_dw, w_pw, b_pw):
    import ml_dtypes
    bf = ml_dtypes.bfloat16
    # wt[p, ec*1024 + dc*128 + j] = w_pw[ec*128+j, dc*128+p]
    wt = np.ascontiguousarray(
        w_pw.reshape(EC, P, DC, P).transpose(3, 0, 2, 1).reshape(P, -1)
    ).astype(bf)
    pp = np.ascontiguousarray(
        np.stack([w_dw[:, 0], w_dw[:, 1], w_dw[:, 2], b_dw, b_pw], axis=1)
        .astype(np.float32).reshape(DC, P, 5).transpose(1, 0, 2)
        .reshape(P, DC * 5))                                     # (P, 40)
    in_maps = []
    for c in range(NCORES):
        b, half = divmod(c, 2)
        l0 = half * LC
        # xpad[d, t]: t 0..1 junk, 2..3 halo (x[l0-2], x[l0-1]), 4.. data
        xpad = np.zeros((D, LC + PAD), dtype=bf)
        lo = max(l0 - 2, 0)
        xpad[:, PAD - (l0 - lo):] = x[b, lo:l0 + LC, :].T.astype(bf)
        xtc = np.empty((P, _XBLK[-1]), dtype=bf)
        for lt, n in enumerate(LTS):
            s = _LT_OFF[lt]
            blk = xpad[:, s:s + n + PAD].reshape(DC, P, n + PAD)
            xtc[:, _XBLK[lt]:_XBLK[lt + 1]] = \
                blk.transpose(1, 0, 2).reshape(P, -1)
        in_maps.append({"xt": xtc, "wt": wt, "pp": pp})
    return in_maps


def kernel(x, w_dw, b_dw, w_pw, b_pw):
    assert x.shape == (B, L, D) and w_dw.shape == (D, KSZ)
    global _CACHED_NC
    if _CACHED_NC is None:
        _CACHED_NC = _build_nc()
    in_maps = _shard_inputs(np.asarray(x, dtype=np.float32),
                            np.asarray(w_dw), np.asarray(b_dw),
                            np.asarray(w_pw), np.asarray(b_pw))
    results = run_bass_kernel_spmd(
        _CACHED_NC, in_maps, list(range(NCORES))).results
    out = np.empty((B, L, D), dtype=np.float32)
    for c in range(NCORES):
        b, half = divmod(c, 2)
        l0 = half * LC
        o = results[c]["ot"]
        for lt, n in enumerate(LTS):
            s = _LT_OFF[lt]
            blk = o[:, EC * s:EC * (s + n)].reshape(P, EC, n)
            out[b, l0 + s:l0 + s + n, :] = \
                blk.transpose(2, 1, 0).reshape(n, D).astype(np.float32)
    return out
